# revision 3
# baseline (speedup 1.0000x reference)
"""Trainium2 Bass kernel for nn_AttentionBlock (dense_cnn) -- v4.

Single fp16 stream of host-pretransposed x feeds both the PE MLP chain
and a DVE weighted-GAP (mul + aligned tree folds + reduce; per-batch
pixels padded 196->208).  The last G=5 channel chunks instead use a
small natural-layout stream reduced on the PE with a masked per-batch
stationary built from K=1 on-chip transposes of a.

Scheduling: on TRN2 engines, semaphore waits are standalone FIFO
instructions -- emission order IS execution order per engine.  The
loop is hand-ordered so every instruction is data-ready when its
engine reaches it: chain(s) -> an-transposes(s) -> pe-gap matmuls(s-1)
-> DVE gap(s-1) (per batch) -> A_all mul(s) -> finalize(s-1).
"""

import numpy as np
from contextlib import ExitStack

from concourse import bacc, mybir, tile
from concourse.bass_utils import run_bass_kernel_spmd

F32 = mybir.dt.float32
F32R = mybir.dt.float32r
BF = mybir.dt.float16
AF = mybir.ActivationFunctionType
AX = mybir.AxisListType
ALU = mybir.AluOpType

B, HH, WW, C = 64, 14, 14, 2048
NCORES = 8
BPC = B // NCORES            # 8 batches per core
PIX = HH * WW                # 196 real pixels per batch
PPAD = 208                   # padded pixels per batch (16*13)
P = 128
NCH = C // P                 # 16 channel chunks
G = 4                        # chunks on the PE-GAP (natural) path
NKD = NCH - G                # 11 chunks on the DVE path
D1, D2, D3 = 64, 16, 8
NPIX = BPC * PPAD            # 1664 padded pixels per core = 13*128
NT = NPIX // P               # 13 natural pixel tiles
NSUP = 4                     # supers of 2 batches
SB = BPC // NSUP             # 2 batches per super
SPX = SB * PPAD              # 416 pixels per super
XT_COLS = NCH * NPIX
TCOV = [[t for t in range(NT) if (P * (t + 1) - 1) // SPX == s]
        for s in range(NSUP)]


def build_program(b4_val: float):
    nc = bacc.Bacc("TRN2", target_bir_lowering=False, debug=False)

    xt_d = nc.dram_tensor("xt", [P, XT_COLS], BF, kind="ExternalInput")
    xn_d = nc.dram_tensor("xn", [NPIX, G * P], BF, kind="ExternalInput")
    w1_d = nc.dram_tensor("W1r", [P, NCH, D1], BF, kind="ExternalInput")
    w2_d = nc.dram_tensor("W2", [D1, D2], BF, kind="ExternalInput")
    w3_d = nc.dram_tensor("W3", [D2, D3], BF, kind="ExternalInput")
    w4_d = nc.dram_tensor("W4", [D3, 2], BF, kind="ExternalInput")
    b1_d = nc.dram_tensor("b1c", [D1, 1], F32, kind="ExternalInput")
    b2_d = nc.dram_tensor("b2c", [D2, 1], F32, kind="ExternalInput")
    b3_d = nc.dram_tensor("b3c", [D3, 1], F32, kind="ExternalInput")
    one_d = nc.dram_tensor("ones1", [1, P], BF, kind="ExternalInput")
    onec_d = nc.dram_tensor("onec", [P, 2], BF, kind="ExternalInput")
    msk_d = nc.dram_tensor("mask8", [P, NT, BPC], BF, kind="ExternalInput")
    idn_d = nc.dram_tensor("idn", [P, P], F32R, kind="ExternalInput")
    out_d = nc.dram_tensor("out", [BPC, C], F32, kind="ExternalOutput")

    with tile.TileContext(nc) as tc, ExitStack() as ctx:
        const = ctx.enter_context(tc.tile_pool(name="const", bufs=1))
        xtp = ctx.enter_context(tc.tile_pool(name="xT", bufs=3))
        xnp = ctx.enter_context(tc.tile_pool(name="xN", bufs=1))
        hpool = ctx.enter_context(tc.tile_pool(name="hsb", bufs=3))
        apool = ctx.enter_context(tc.tile_pool(name="absb", bufs=3))
        tmpp = ctx.enter_context(tc.tile_pool(name="tmp", bufs=2))
        trees = ctx.enter_context(tc.tile_pool(name="tree", bufs=2))
        acc = ctx.enter_context(tc.tile_pool(name="acc", bufs=1))
        ps_h1 = ctx.enter_context(tc.tile_pool(name="h1ps", bufs=2, space="PSUM"))
        ps_sm = ctx.enter_context(tc.tile_pool(name="smps", bufs=1, space="PSUM"))
        ps_ab = ctx.enter_context(tc.tile_pool(name="abps", bufs=1, space="PSUM"))
        ps_an = ctx.enter_context(tc.tile_pool(name="anps", bufs=1, space="PSUM"))
        ps_gp = ctx.enter_context(tc.tile_pool(name="gpps", bufs=1, space="PSUM"))
        ps_cnt = ctx.enter_context(tc.tile_pool(name="cntps", bufs=1, space="PSUM"))
        ps_fin = ctx.enter_context(tc.tile_pool(name="finps", bufs=1, space="PSUM"))

        # ---- warm the ACT function tables FIRST: the table loads ride
        # the scalar queue before any DMA-descriptor issue ----
        warm = acc.tile([1, 2], F32)
        nc.vector.memset(warm[:], 0.0)
        wo = acc.tile([1, 2], BF)
        nc.scalar.activation(wo[:], warm[:], AF.Relu, bias=0.0)
        nc.scalar.activation(wo[:], warm[:], AF.Sigmoid, bias=b4_val)

        # ---- constants on the scalar HWDGE queue ----
        w1_sb = const.tile([P, NCH, D1], BF)
        nc.scalar.dma_start(w1_sb[:], w1_d[:])
        w2_sb = const.tile([D1, D2], BF)
        nc.scalar.dma_start(w2_sb[:], w2_d[:])
        w3_sb = const.tile([D2, D3], BF)
        nc.scalar.dma_start(w3_sb[:], w3_d[:])
        w4_sb = const.tile([D3, 2], BF)
        nc.scalar.dma_start(w4_sb[:], w4_d[:])
        b1_sb = const.tile([D1, 1], F32)
        nc.scalar.dma_start(b1_sb[:], b1_d[:])
        b2_sb = const.tile([D2, 1], F32)
        nc.scalar.dma_start(b2_sb[:], b2_d[:])
        b3_sb = const.tile([D3, 1], F32)
        nc.scalar.dma_start(b3_sb[:], b3_d[:])
        ones1 = const.tile([1, P], BF)
        nc.scalar.dma_start(ones1[:], one_d[:])
        onec = const.tile([P, 2], BF)
        nc.scalar.dma_start(onec[:], onec_d[:])
        mask8 = const.tile([P, NT, BPC], BF)
        nc.scalar.dma_start(mask8[:], msk_d[:])
        idn = const.tile([P, P], F32R)
        nc.scalar.dma_start(idn[:], idn_d[:])
        xn_sb = xnp.tile([P, NT, G * P], BF)
        xnv = xn_d[:].rearrange("(t p) c -> p t c", t=NT)

        # ---- warm the PE clock with dependency-free junk matmuls on a
        # never-written tile: they issue right after the preamble, so the
        # HAM is at 2.4 GHz before the first real h1 ----
        junk = const.tile([P, SPX], BF)
        nc.gpsimd.memset(junk[:], 0.0)
        wps = ps_ab.tile([P, SPX], F32, tag="ab")
        for _ in range(9):
            nc.tensor.matmul(wps[0:D1, :], junk[:, 0:D1], junk[:],
                             start=True, stop=True)

        # ---- long-lived state ----
        gapT = acc.tile([P, BPC, NKD], F32)
        cnt = acc.tile([P, BPC], F32)
        aT_all = acc.tile([1, NPIX], BF)
        A_all = acc.tile([P, NT, BPC], BF)
        an_ps = ps_an.tile([P, NT], F32, tag="an")
        gp_ps = ps_gp.tile([BPC, G * P], F32, tag="gp")
        cnt_ps = ps_cnt.tile([BPC, 2], F32, tag="cntp")
        nc.vector.memset(gp_ps[:], 0.0)
        nc.vector.memset(cnt_ps[:], 0.0)

        xts = [None] * NSUP

        def emit_xt(s):
            t = xtp.tile([P, NCH, SPX], BF, tag="xt")
            c0 = NCH * SPX * s
            q = NCH // 4
            for h in range(4):
                nc.sync.dma_start(
                    t[:, q * h:q * (h + 1), :].rearrange("p k s -> p (k s)"),
                    xt_d[:, c0 + q * h * SPX:c0 + q * (h + 1) * SPX])
            xts[s] = t

        emit_xt(0)
        emit_xt(1)

        def h1_head(s, half):
            xt = xts[s]
            if half == 0:
                t = ps_h1.tile([D1, SPX], F32, tag="h1")
                h1_head.ps[s] = t
            h1_ps = h1_head.ps[s]
            for k in range(8 * half, 8 * (half + 1)):
                nc.tensor.matmul(h1_ps[:], w1_sb[:, k, :], xt[:, k, :],
                                 start=(k == 0), stop=(k == NCH - 1))
        h1_head.ps = [None] * NSUP

        def chain_tail(s, interleave=None):
            """h2..a_bc for super s; `interleave` emits h1(s+1) halves
            into the PE's relu-wait bubbles."""
            h1_sb = hpool.tile([D1, SPX], BF, tag="h1")
            nc.scalar.activation(h1_sb[:], h1_head.ps[s][:], AF.Relu,
                                 bias=b1_sb[:])
            if interleave is not None:
                h1_head(interleave, 0)
            h2_ps = ps_sm.tile([D2, SPX], F32, tag="sm")
            nc.tensor.matmul(h2_ps[:], w2_sb[:], h1_sb[:], start=True, stop=True)
            h2_sb = hpool.tile([D2, SPX], BF, tag="h2")
            nc.scalar.activation(h2_sb[:], h2_ps[:], AF.Relu, bias=b2_sb[:])
            if interleave is not None:
                h1_head(interleave, 1)
            h3_ps = ps_sm.tile([D3, SPX], F32, tag="sm")
            nc.tensor.matmul(h3_ps[:], w3_sb[:], h2_sb[:], start=True, stop=True)
            h3_sb = hpool.tile([D3, SPX], BF, tag="h3")
            nc.scalar.activation(h3_sb[:], h3_ps[:], AF.Relu, bias=b3_sb[:])
            a_ps = ps_sm.tile([2, SPX], F32, tag="sm")
            nc.tensor.matmul(a_ps[:], w4_sb[:], h3_sb[:], start=True, stop=True)
            aT = aT_all[0:1, SPX * s:SPX * (s + 1)]
            nc.scalar.activation(aT, a_ps[0:1, :], AF.Sigmoid, bias=b4_val)
            if s > 0:
                gp_mms(s - 1)
            ab_ps = ps_ab.tile([P, SPX], F32, tag="ab")
            nc.tensor.matmul(ab_ps[:], ones1[:], aT, start=True, stop=True)
            a_bc = apool.tile([P, SPX], BF, tag="abc")
            for b in range(SB):
                o = PPAD * b
                nc.scalar.activation(
                    a_bc[:, o:o + PIX], ab_ps[:, o:o + PIX], AF.Copy,
                    accum_out=cnt[:, SB * s + b:SB * s + b + 1])
                nc.scalar.activation(
                    a_bc[:, o + PIX:o + PPAD], ab_ps[:, o + PIX:o + PPAD],
                    AF.Copy)
            return a_bc

        def an_k1(s):
            for t in TCOV[s]:
                nc.tensor.matmul(an_ps[:, t:t + 1],
                                 aT_all[0:1, P * t:P * (t + 1)],
                                 onec[0:1, 0:1], start=True, stop=True)

        def a_mul(s):
            tl = TCOV[s]
            t0, t1 = tl[0], tl[-1] + 1
            nc.vector.tensor_mul(
                A_all[:, t0:t1, :],
                an_ps[:, t0:t1].unsqueeze(2).broadcast_to([P, t1 - t0, BPC]),
                mask8[:, t0:t1, :])

        def gp_mms(s):
            for t in TCOV[s]:
                nc.tensor.matmul(gp_ps[:, 0:G * P], A_all[:, t, :],
                                 xn_sb[:, t, :], start=False,
                                 stop=(t == NT - 1))
                nc.tensor.matmul(cnt_ps[:], A_all[:, t, :],
                                 onec[:], start=False, stop=(t == NT - 1))

        def gap(s, a_bc, blist=None):
            xt = xts[s]
            tmp = tmpp.tile([P, NKD, SPX], BF, tag="tmp")
            for b in range(SB):
                o = PPAD * b
                sl = slice(o, o + PPAD)
                nc.vector.tensor_mul(
                    tmp[:, :, sl], xt[:, 0:NKD, sl],
                    a_bc[:, sl].unsqueeze(1).broadcast_to([P, NKD, PPAD]))
                t1 = trees.tile([P, NKD, 104], BF, tag="t1")
                v = tmp[:, :, sl]
                nc.vector.tensor_add(t1[:], v[:, :, 0:104], v[:, :, 104:208])
                t2 = trees.tile([P, NKD, 52], BF, tag="t2")
                nc.vector.tensor_add(t2[:], t1[:, :, 0:52], t1[:, :, 52:104])
                t3 = trees.tile([P, NKD, 26], BF, tag="t3")
                nc.vector.tensor_add(t3[:], t2[:, :, 0:26], t2[:, :, 26:52])
                nc.vector.tensor_reduce(
                    gapT[:, SB * s + b, :], t3[:], axis=AX.X, op=ALU.add)

        gscs = [None] * NSUP

        def finalize_dve(s):
            rs = acc.tile([P, SB], F32, tag="rs")
            nc.vector.reciprocal(rs[:], cnt[:, SB * s:SB * (s + 1)])
            gsc = acc.tile([P, SB * NKD], F32R, tag=f"gsc{s % 2}")
            nc.vector.tensor_mul(
                gsc[:].rearrange("p (b k) -> p b k", b=SB),
                gapT[:, SB * s:SB * (s + 1), :],
                rs[:].unsqueeze(2).broadcast_to([P, SB, NKD]))
            gscs[s] = gsc

        def finalize_pe(s):
            """Deferred a full super so the transpose never blocks the
            PE FIFO on the DVE block."""
            fin = ps_fin.tile([SB * NKD, P], F32R, tag="fin")
            nc.tensor.transpose(fin[:], gscs[s][:], idn[:])
            osb = acc.tile([SB * NKD, P], F32, tag="osb")
            nc.scalar.activation(osb[:], fin[:], AF.Copy)
            for b in range(SB):
                nc.sync.dma_start(
                    out_d[SB * s + b:SB * s + b + 1, 0:NKD * P]
                    .rearrange("o (k c) -> (o k) c", k=NKD),
                    osb[NKD * b:NKD * (b + 1), :])

        # ---- hand-scheduled software pipeline ----
        prev = None
        h1_head(0, 0)
        h1_head(0, 1)
        for s in range(NSUP):
            if s + 2 < NSUP:
                emit_xt(s + 2)
            if s < 2:
                h = NT // 2 + 1
                sl = slice(0, h) if s == 0 else slice(h, NT)
                nc.sync.dma_start(xn_sb[:, sl, :], xnv[:, sl, :])
            a_bc = chain_tail(s, interleave=s + 1 if s + 1 < NSUP else None)
            an_k1(s)
            if s >= 2:
                finalize_pe(s - 2)
            if prev is not None:
                gap(s - 1, prev)
            a_mul(s)
            if prev is not None:
                finalize_dve(s - 1)
            prev = a_bc
        gp_mms(NSUP - 1)
        finalize_pe(NSUP - 2)
        gap(NSUP - 1, prev)
        finalize_dve(NSUP - 1)
        finalize_pe(NSUP - 1)

        # ---- PE-path channels: scale by 1/cnt (partition==batch) ----
        rgp = acc.tile([BPC, 1], F32)
        nc.vector.reciprocal(rgp[:], cnt_ps[:, 0:1])
        ogp = acc.tile([BPC, G * P], F32)
        nc.scalar.activation(ogp[:], gp_ps[:], AF.Copy, scale=rgp[:])
        nc.sync.dma_start(out_d[:, NKD * P:C], ogp[:])

    nc.compile()
    return nc


def make_in_maps(x, W1, b1, W2, b2, W3, b3, W4, b4):
    x = np.asarray(x, dtype=np.float32)
    mask = np.zeros((P, NT, BPC), dtype=np.float16)
    for t in range(NT):
        for p in range(P):
            pix = t * P + p
            if pix % PPAD < PIX:
                mask[p, t, pix // PPAD] = 1.0
    base = {
        "W1r": np.ascontiguousarray(
            np.asarray(W1, np.float32).reshape(NCH, P, D1).transpose(1, 0, 2)
            .astype(np.float16)),
        "W2": np.ascontiguousarray(np.asarray(W2, np.float16)),
        "W3": np.ascontiguousarray(np.asarray(W3, np.float16)),
        "W4": np.ascontiguousarray(np.concatenate(
            [np.asarray(W4, np.float32),
             np.zeros((D3, 1), np.float32)], axis=1).astype(np.float16)),
        "b1c": np.asarray(b1, np.float32).reshape(D1, 1).copy(),
        "b2c": np.asarray(b2, np.float32).reshape(D2, 1).copy(),
        "b3c": np.asarray(b3, np.float32).reshape(D3, 1).copy(),
        "ones1": np.ones((1, P), dtype=np.float16),
        "onec": np.ones((P, 2), dtype=np.float16),
        "mask8": mask,
        "idn": np.eye(P, dtype=np.float32),
    }
    xs = x.reshape(B, PIX, C)
    maps = []
    for c in range(NCORES):
        xp = np.zeros((BPC, PPAD, C), dtype=np.float32)
        xp[:, :PIX] = xs[c * BPC:(c + 1) * BPC]
        xf = xp.reshape(NPIX, C)
        xt3 = xf.T.reshape(NCH, P, NPIX).transpose(1, 0, 2)
        blocks = [xt3[:, :, SPX * s:SPX * (s + 1)].reshape(P, -1)
                  for s in range(NSUP)]
        xct = np.ascontiguousarray(
            np.concatenate(blocks, axis=1)).astype(np.float16)
        xn = np.ascontiguousarray(xf[:, NKD * P:C]).astype(np.float16)
        maps.append({"xt": xct, "xn": xn, **base})
    return maps


def kernel(x, W1, b1, W2, b2, W3, b3, W4, b4, _profile=False, **_ignored):
    nc = build_program(float(np.asarray(b4, np.float32).reshape(-1)[0]))
    in_maps = make_in_maps(x, W1, b1, W2, b2, W3, b3, W4, b4)
    res = run_bass_kernel_spmd(nc, in_maps, core_ids=list(range(NCORES)),
                               trace=_profile)
    out = np.concatenate([res.results[c]["out"] for c in range(NCORES)], axis=0)
    out = np.ascontiguousarray(out.astype(np.float32))
    if _profile:
        return out, res
    return out


# revision 4
# speedup vs baseline: 1.1494x; 1.1494x over previous
"""Trainium2 Bass kernel for nn_AttentionBlock (dense_cnn) -- v4.

Single fp16 stream of host-pretransposed x feeds both the PE MLP chain
and a DVE weighted-GAP (mul + aligned tree folds + reduce; per-batch
pixels padded 196->208).  The last G=5 channel chunks instead use a
small natural-layout stream reduced on the PE with a masked per-batch
stationary built from K=1 on-chip transposes of a.

Scheduling: on TRN2 engines, semaphore waits are standalone FIFO
instructions -- emission order IS execution order per engine.  The
loop is hand-ordered so every instruction is data-ready when its
engine reaches it: chain(s) -> an-transposes(s) -> pe-gap matmuls(s-1)
-> DVE gap(s-1) (per batch) -> A_all mul(s) -> finalize(s-1).
"""

import numpy as np
from contextlib import ExitStack

from concourse import bacc, mybir, tile
from concourse.bass_utils import run_bass_kernel_spmd

F32 = mybir.dt.float32
F32R = mybir.dt.float32r
BF = mybir.dt.float16
AF = mybir.ActivationFunctionType
AX = mybir.AxisListType
ALU = mybir.AluOpType

B, HH, WW, C = 64, 14, 14, 2048
NCORES = 8
BPC = B // NCORES            # 8 batches per core
PIX = HH * WW                # 196 real pixels per batch
PPAD = 208                   # padded pixels per batch (16*13)
P = 128
NCH = C // P                 # 16 channel chunks
G = 4                        # chunks on the PE-GAP (natural) path
NKD = NCH - G                # 11 chunks on the DVE path
D1, D2, D3 = 64, 16, 8
NPIX = BPC * PPAD            # 1664 padded pixels per core = 13*128
NT = NPIX // P               # 13 natural pixel tiles
NSUP = 4                     # supers of 2 batches
SB = BPC // NSUP             # 2 batches per super
SPX = SB * PPAD              # 416 pixels per super
XT_COLS = NCH * NPIX
TCOV = [[t for t in range(NT) if (P * (t + 1) - 1) // SPX == s]
        for s in range(NSUP)]


def build_program(b4_val: float):
    nc = bacc.Bacc("TRN2", target_bir_lowering=False, debug=False)

    xt_d = nc.dram_tensor("xt", [P, XT_COLS], BF, kind="ExternalInput")
    xn_d = nc.dram_tensor("xn", [NPIX, G * P], BF, kind="ExternalInput")
    w1_d = nc.dram_tensor("W1r", [P, NCH, D1], BF, kind="ExternalInput")
    w2_d = nc.dram_tensor("W2", [D1, D2], BF, kind="ExternalInput")
    w3_d = nc.dram_tensor("W3", [D2, D3], BF, kind="ExternalInput")
    w4_d = nc.dram_tensor("W4", [D3, 2], BF, kind="ExternalInput")
    b1_d = nc.dram_tensor("b1c", [D1, 1], F32, kind="ExternalInput")
    b2_d = nc.dram_tensor("b2c", [D2, 1], F32, kind="ExternalInput")
    b3_d = nc.dram_tensor("b3c", [D3, 1], F32, kind="ExternalInput")
    one_d = nc.dram_tensor("ones1", [1, P], BF, kind="ExternalInput")
    onec_d = nc.dram_tensor("onec", [P, 2], BF, kind="ExternalInput")
    msk_d = nc.dram_tensor("mask8", [P, NT, BPC], BF, kind="ExternalInput")
    idn_d = nc.dram_tensor("idn", [P, P], F32R, kind="ExternalInput")
    out_d = nc.dram_tensor("out", [BPC, C], F32, kind="ExternalOutput")

    with tile.TileContext(nc) as tc, ExitStack() as ctx:
        const = ctx.enter_context(tc.tile_pool(name="const", bufs=1))
        xtp = ctx.enter_context(tc.tile_pool(name="xT", bufs=3))
        xnp = ctx.enter_context(tc.tile_pool(name="xN", bufs=1))
        hpool = ctx.enter_context(tc.tile_pool(name="hsb", bufs=3))
        apool = ctx.enter_context(tc.tile_pool(name="absb", bufs=3))
        tmpp = ctx.enter_context(tc.tile_pool(name="tmp", bufs=2))
        trees = ctx.enter_context(tc.tile_pool(name="tree", bufs=2))
        acc = ctx.enter_context(tc.tile_pool(name="acc", bufs=1))
        ps_h1 = ctx.enter_context(tc.tile_pool(name="h1ps", bufs=2, space="PSUM"))
        ps_sm = ctx.enter_context(tc.tile_pool(name="smps", bufs=1, space="PSUM"))
        ps_ab = ctx.enter_context(tc.tile_pool(name="abps", bufs=1, space="PSUM"))
        ps_an = ctx.enter_context(tc.tile_pool(name="anps", bufs=1, space="PSUM"))
        ps_gp = ctx.enter_context(tc.tile_pool(name="gpps", bufs=1, space="PSUM"))
        ps_cnt = ctx.enter_context(tc.tile_pool(name="cntps", bufs=1, space="PSUM"))
        ps_fin = ctx.enter_context(tc.tile_pool(name="finps", bufs=1, space="PSUM"))

        # ---- warm the ACT function tables FIRST: the table loads ride
        # the scalar queue before any DMA-descriptor issue ----
        warm = acc.tile([1, 2], F32)
        nc.vector.memset(warm[:], 0.0)
        wo = acc.tile([1, 2], BF)
        nc.scalar.activation(wo[:], warm[:], AF.Relu, bias=0.0)
        nc.scalar.activation(wo[:], warm[:], AF.Sigmoid, bias=b4_val)

        # ---- constants on the scalar HWDGE queue ----
        w1_sb = const.tile([P, NCH, D1], BF)
        nc.scalar.dma_start(w1_sb[:], w1_d[:])
        w2_sb = const.tile([D1, D2], BF)
        nc.scalar.dma_start(w2_sb[:], w2_d[:])
        w3_sb = const.tile([D2, D3], BF)
        nc.scalar.dma_start(w3_sb[:], w3_d[:])
        w4_sb = const.tile([D3, 2], BF)
        nc.scalar.dma_start(w4_sb[:], w4_d[:])
        b1_sb = const.tile([D1, 1], F32)
        nc.scalar.dma_start(b1_sb[:], b1_d[:])
        b2_sb = const.tile([D2, 1], F32)
        nc.scalar.dma_start(b2_sb[:], b2_d[:])
        b3_sb = const.tile([D3, 1], F32)
        nc.scalar.dma_start(b3_sb[:], b3_d[:])
        ones1 = const.tile([1, P], BF)
        nc.scalar.dma_start(ones1[:], one_d[:])
        onec = const.tile([P, 2], BF)
        nc.scalar.dma_start(onec[:], onec_d[:])
        mask8 = const.tile([P, NT, BPC], BF)
        nc.scalar.dma_start(mask8[:], msk_d[:])
        idn = const.tile([P, P], F32R)
        nc.scalar.dma_start(idn[:], idn_d[:])
        xn_sb = xnp.tile([P, NT, G * P], BF)
        xnv = xn_d[:].rearrange("(t p) c -> p t c", t=NT)

        # ---- warm the PE clock with dependency-free junk matmuls on a
        # never-written tile: they issue right after the preamble, so the
        # HAM is at 2.4 GHz before the first real h1 ----
        junk = const.tile([P, SPX], BF)
        nc.gpsimd.memset(junk[:], 0.0)
        wps = ps_ab.tile([P, SPX], F32, tag="ab")
        for _ in range(9):
            nc.tensor.matmul(wps[0:D1, :], junk[:, 0:D1], junk[:],
                             start=True, stop=True)

        # ---- long-lived state ----
        gapT = acc.tile([P, BPC, NKD], F32)
        cnt = acc.tile([P, BPC], F32)
        aT_all = acc.tile([1, NPIX], BF)
        A_all = acc.tile([P, NT, BPC], BF)
        an_ps = ps_an.tile([P, NT], F32, tag="an")
        gp_ps = ps_gp.tile([BPC, G * P], F32, tag="gp")
        cnt_ps = ps_cnt.tile([BPC, 2], F32, tag="cntp")
        nc.vector.memset(gp_ps[:], 0.0)
        nc.vector.memset(cnt_ps[:], 0.0)

        xts = [None] * NSUP

        def emit_xt(s):
            t = xtp.tile([P, NCH, SPX], BF, tag="xt")
            c0 = NCH * SPX * s
            q = NCH // 4
            for h in range(4):
                nc.sync.dma_start(
                    t[:, q * h:q * (h + 1), :].rearrange("p k s -> p (k s)"),
                    xt_d[:, c0 + q * h * SPX:c0 + q * (h + 1) * SPX])
            xts[s] = t

        emit_xt(0)
        emit_xt(1)

        def h1_head(s, half):
            xt = xts[s]
            if half == 0:
                t = ps_h1.tile([D1, SPX], F32, tag="h1")
                h1_head.ps[s] = t
            h1_ps = h1_head.ps[s]
            for k in range(8 * half, 8 * (half + 1)):
                nc.tensor.matmul(h1_ps[:], w1_sb[:, k, :], xt[:, k, :],
                                 start=(k == 0), stop=(k == NCH - 1))
        h1_head.ps = [None] * NSUP

        def chain_tail(s, interleave=None):
            """h2..a_bc for super s; `interleave` emits h1(s+1) halves
            into the PE's relu-wait bubbles."""
            h1_sb = hpool.tile([D1, SPX], BF, tag="h1")
            nc.scalar.activation(h1_sb[:], h1_head.ps[s][:], AF.Relu,
                                 bias=b1_sb[:])
            if interleave is not None:
                h1_head(interleave, 0)
            h2_ps = ps_sm.tile([D2, SPX], F32, tag="sm")
            nc.tensor.matmul(h2_ps[:], w2_sb[:], h1_sb[:], start=True, stop=True)
            h2_sb = hpool.tile([D2, SPX], BF, tag="h2")
            nc.scalar.activation(h2_sb[:], h2_ps[:], AF.Relu, bias=b2_sb[:])
            if interleave is not None:
                h1_head(interleave, 1)
            h3_ps = ps_sm.tile([D3, SPX], F32, tag="sm")
            nc.tensor.matmul(h3_ps[:], w3_sb[:], h2_sb[:], start=True, stop=True)
            h3_sb = hpool.tile([D3, SPX], BF, tag="h3")
            nc.scalar.activation(h3_sb[:], h3_ps[:], AF.Relu, bias=b3_sb[:])
            a_ps = ps_sm.tile([2, SPX], F32, tag="sm")
            nc.tensor.matmul(a_ps[:], w4_sb[:], h3_sb[:], start=True, stop=True)
            aT = aT_all[0:1, SPX * s:SPX * (s + 1)]
            nc.scalar.activation(aT, a_ps[0:1, :], AF.Sigmoid, bias=b4_val)
            if s > 0:
                gp_mms(s - 1)
            ab_ps = ps_ab.tile([P, SPX], F32, tag="ab")
            nc.tensor.matmul(ab_ps[:], ones1[:], aT, start=True, stop=True)
            a_bc = apool.tile([P, SPX], BF, tag="abc")
            for b in range(SB):
                o = PPAD * b
                nc.scalar.activation(
                    a_bc[:, o:o + PIX], ab_ps[:, o:o + PIX], AF.Copy,
                    accum_out=cnt[:, SB * s + b:SB * s + b + 1])
                nc.scalar.activation(
                    a_bc[:, o + PIX:o + PPAD], ab_ps[:, o + PIX:o + PPAD],
                    AF.Copy)
            return a_bc

        def an_k1(s):
            for t in TCOV[s]:
                nc.tensor.matmul(an_ps[:, t:t + 1],
                                 aT_all[0:1, P * t:P * (t + 1)],
                                 onec[0:1, 0:1], start=True, stop=True)

        def a_mul(s):
            tl = TCOV[s]
            t0, t1 = tl[0], tl[-1] + 1
            nc.vector.tensor_mul(
                A_all[:, t0:t1, :],
                an_ps[:, t0:t1].unsqueeze(2).broadcast_to([P, t1 - t0, BPC]),
                mask8[:, t0:t1, :])

        def gp_mms(s):
            for t in TCOV[s]:
                nc.tensor.matmul(gp_ps[:, 0:G * P], A_all[:, t, :],
                                 xn_sb[:, t, :], start=False,
                                 stop=(t == NT - 1))
                nc.tensor.matmul(cnt_ps[:], A_all[:, t, :],
                                 onec[:], start=False, stop=(t == NT - 1))

        def gap(s, a_bc, blist=None):
            xt = xts[s]
            tmp = tmpp.tile([P, NKD, SPX], BF, tag="tmp")
            for b in range(SB):
                o = PPAD * b
                sl = slice(o, o + PPAD)
                nc.vector.tensor_mul(
                    tmp[:, :, sl], xt[:, 0:NKD, sl],
                    a_bc[:, sl].unsqueeze(1).broadcast_to([P, NKD, PPAD]))
                t1 = trees.tile([P, NKD, 104], BF, tag="t1")
                v = tmp[:, :, sl]
                nc.vector.tensor_add(t1[:], v[:, :, 0:104], v[:, :, 104:208])
                t2 = trees.tile([P, NKD, 52], BF, tag="t2")
                nc.vector.tensor_add(t2[:], t1[:, :, 0:52], t1[:, :, 52:104])
                t3 = trees.tile([P, NKD, 26], BF, tag="t3")
                nc.vector.tensor_add(t3[:], t2[:, :, 0:26], t2[:, :, 26:52])
                nc.vector.tensor_reduce(
                    gapT[:, SB * s + b, :], t3[:], axis=AX.X, op=ALU.add)

        gscs = [None] * NSUP

        def finalize_dve(s):
            rs = acc.tile([P, SB], F32, tag="rs")
            nc.vector.reciprocal(rs[:], cnt[:, SB * s:SB * (s + 1)])
            gsc = acc.tile([P, SB * NKD], F32R, tag=f"gsc{s % 2}")
            nc.vector.tensor_mul(
                gsc[:].rearrange("p (b k) -> p b k", b=SB),
                gapT[:, SB * s:SB * (s + 1), :],
                rs[:].unsqueeze(2).broadcast_to([P, SB, NKD]))
            gscs[s] = gsc

        def finalize_pe(s):
            """Deferred a full super so the transpose never blocks the
            PE FIFO on the DVE block."""
            fin = ps_fin.tile([SB * NKD, P], F32R, tag="fin")
            nc.tensor.transpose(fin[:], gscs[s][:], idn[:])
            osb = acc.tile([SB * NKD, P], F32, tag="osb")
            nc.scalar.activation(osb[:], fin[:], AF.Copy)
            for b in range(SB):
                nc.sync.dma_start(
                    out_d[SB * s + b:SB * s + b + 1, 0:NKD * P]
                    .rearrange("o (k c) -> (o k) c", k=NKD),
                    osb[NKD * b:NKD * (b + 1), :])

        # ---- hand-scheduled software pipeline ----
        prev = None
        h1_head(0, 0)
        h1_head(0, 1)
        for s in range(NSUP):
            if s + 2 < NSUP:
                emit_xt(s + 2)
            if s < 2:
                h = NT // 2 + 1
                sl = slice(0, h) if s == 0 else slice(h, NT)
                nc.sync.dma_start(xn_sb[:, sl, :], xnv[:, sl, :])
            a_bc = chain_tail(s, interleave=s + 1 if s + 1 < NSUP else None)
            an_k1(s)
            if s >= 2:
                finalize_pe(s - 2)
            if prev is not None:
                gap(s - 1, prev)
            a_mul(s)
            if prev is not None:
                finalize_dve(s - 1)
            prev = a_bc
        gp_mms(NSUP - 1)
        finalize_pe(NSUP - 2)
        # PE-path channels finalize BEFORE the last DVE block: its tiny
        # reciprocal must not sit behind gap(3) in the DVE FIFO
        rgp = acc.tile([BPC, 1], F32)
        nc.vector.reciprocal(rgp[:], cnt_ps[:, 0:1])
        ogp = acc.tile([BPC, G * P], F32)
        nc.scalar.activation(ogp[:], gp_ps[:], AF.Copy, scale=rgp[:])
        nc.sync.dma_start(out_d[:, NKD * P:C], ogp[:])
        gap(NSUP - 1, prev)
        finalize_dve(NSUP - 1)
        finalize_pe(NSUP - 1)

    nc.compile()
    return nc


def make_in_maps(x, W1, b1, W2, b2, W3, b3, W4, b4):
    x = np.asarray(x, dtype=np.float32)
    mask = np.zeros((P, NT, BPC), dtype=np.float16)
    for t in range(NT):
        for p in range(P):
            pix = t * P + p
            if pix % PPAD < PIX:
                mask[p, t, pix // PPAD] = 1.0
    base = {
        "W1r": np.ascontiguousarray(
            np.asarray(W1, np.float32).reshape(NCH, P, D1).transpose(1, 0, 2)
            .astype(np.float16)),
        "W2": np.ascontiguousarray(np.asarray(W2, np.float16)),
        "W3": np.ascontiguousarray(np.asarray(W3, np.float16)),
        "W4": np.ascontiguousarray(np.concatenate(
            [np.asarray(W4, np.float32),
             np.zeros((D3, 1), np.float32)], axis=1).astype(np.float16)),
        "b1c": np.asarray(b1, np.float32).reshape(D1, 1).copy(),
        "b2c": np.asarray(b2, np.float32).reshape(D2, 1).copy(),
        "b3c": np.asarray(b3, np.float32).reshape(D3, 1).copy(),
        "ones1": np.ones((1, P), dtype=np.float16),
        "onec": np.ones((P, 2), dtype=np.float16),
        "mask8": mask,
        "idn": np.eye(P, dtype=np.float32),
    }
    xs = x.reshape(B, PIX, C)
    maps = []
    for c in range(NCORES):
        xp = np.zeros((BPC, PPAD, C), dtype=np.float32)
        xp[:, :PIX] = xs[c * BPC:(c + 1) * BPC]
        xf = xp.reshape(NPIX, C)
        xt3 = xf.T.reshape(NCH, P, NPIX).transpose(1, 0, 2)
        blocks = [xt3[:, :, SPX * s:SPX * (s + 1)].reshape(P, -1)
                  for s in range(NSUP)]
        xct = np.ascontiguousarray(
            np.concatenate(blocks, axis=1)).astype(np.float16)
        xn = np.ascontiguousarray(xf[:, NKD * P:C]).astype(np.float16)
        maps.append({"xt": xct, "xn": xn, **base})
    return maps


def kernel(x, W1, b1, W2, b2, W3, b3, W4, b4, _profile=False, **_ignored):
    nc = build_program(float(np.asarray(b4, np.float32).reshape(-1)[0]))
    in_maps = make_in_maps(x, W1, b1, W2, b2, W3, b3, W4, b4)
    res = run_bass_kernel_spmd(nc, in_maps, core_ids=list(range(NCORES)),
                               trace=_profile)
    out = np.concatenate([res.results[c]["out"] for c in range(NCORES)], axis=0)
    out = np.ascontiguousarray(out.astype(np.float32))
    if _profile:
        return out, res
    return out
